# revision 15
# baseline (speedup 1.0000x reference)
"""Trainium2 Bass kernel for nn_AwesomeGRU (SEQ=512, B=64, DIM=1024, UNITS=1024).

Algorithm: the `reset` input zeroes h *before* each masked step, so each batch
row's recurrence splits into independent segments (h carries over only within
a segment). Classic packed-sequence reformulation:

  host: enumerate segments, sort by length desc, deal round-robin to 8 cores,
        lay tokens out depth-major ((depth, segment-rank) order). Pass j
        processes all tokens at depth j — a contiguous row block whose h
        inputs are a PREFIX of pass j-1's outputs (no gather).
  core: for each pass j: PSUM <- x_j @ W_ih^T (+ h_j @ W_hh^T if j>0), then
        gates elementwise, h -> fp16 SBUF buffer that doubles as the DMA-out
        source and pass j+1's matmul input.
  host: inverse-permute output tokens to (seq, b, units).

Everything is feature-major on device: activations stored (units, rows) so no
transposes are ever needed. The r-gate matmuls run in fp8 e4m3 DoubleRow mode
(2x PE rate; weights stored *4096 to stay in normal range, rescale folded
into the sigmoid's scale); the error-analysis margin allows fp8 only on r
(its error is damped by sigmoid' <= 1/4 and only reaches h through tanh).
z/n matmuls are fp16 (same PE rate as bf16, 3 more mantissa bits). PSUM
accumulates fp32; elementwise fp16. Depth-0 tokens (h=0) skip the h-matmul.

Deep passes (m <= TAILM rows) pack all 8 unit-groups into one PSUM bank per
gate, with per-unit biases injected via K=1 matmuls against a ones vector, so
the whole gate pipeline is ~10 wide instructions instead of ~64 narrow ones.
The gi presweep for deep rows is emitted in per-pass blocks three passes
ahead of use: independent PE work that fills the inter-pass dependency
bubbles (PE p-state ramping makes every bubble cost ~2x its length).

Self-contained: derives everything from the runtime value of `reset`.
"""
import os
import numpy as np
import ml_dtypes

import concourse.bacc as bacc
import concourse.mybir as mybir
import concourse.tile as tile
from concourse.bass_utils import run_bass_kernel_spmd

SEQ, B, DIM, UNITS = 512, 64, 1024, 1024
NCORES = 8
P = 128
CG = DIM // P        # 8 contraction groups per matmul side
PG = CG // 2         # 4 fp8 DoubleRow pair-groups (2x128 contraction each)
UG = UNITS // P      # 8 unit groups
CH = 512             # row-chunk (free dim / PSUM bank)
TAILM = 32           # packed-tail threshold: 8 unit groups fit one PSUM bank
dt = mybir.dt
f32 = dt.float32
bf16 = dt.float16  # fp16: same PE rate as bf16, 3 more mantissa bits
fp8 = dt.float8e4  # e4m3: r-gate matmuls in DoubleRow mode (2x PE rate)
W8SCALE = 4096.0   # fp8 weights stored *4096 (else all subnormal); ps_r scaled

LAST_EXEC_NS = None  # set when GRU_TRACE=1


# ---------------------------------------------------------------- host plan

def _build_plan(reset_sb, h0_any):
    """reset_sb: (SEQ, B) bool. Returns (m_j schedule, per-core token maps).

    Segment starts: t=0 always (h0 seed row: h0[b] unless reset[0,b]), and
    every t>0 with reset=1 (h zeroed exactly).
    """
    segs = []  # (length, b, t_start)
    for b in range(B):
        col = reset_sb[:, b]
        starts = [0] + [t for t in range(1, SEQ) if col[t]]
        for i, s in enumerate(starts):
            e = starts[i + 1] if i + 1 < len(starts) else SEQ
            segs.append((e - s, b, s))
    segs.sort(key=lambda x: (-x[0], x[1], x[2]))
    Lmax = segs[0][0]
    n_j = [0] * Lmax
    for L, _, _ in segs:
        for j in range(L):
            n_j[j] += 1
    m_j = [(n + NCORES - 1) // NCORES for n in n_j]

    plans = []
    for c in range(NCORES):
        mysegs = segs[c::NCORES]
        tok = np.full(sum(m_j), -1, np.int64)  # flat t*B+b index or -1 pad
        seed_b = np.full(m_j[0], -1, np.int64)  # batch row for h seed (pass 0)
        off = 0
        for j in range(Lmax):
            for r in range(m_j[j]):
                if r < len(mysegs) and mysegs[r][0] > j:
                    L, b, s = mysegs[r]
                    tok[off + r] = (s + j) * B + b
                    if j == 0 and s == 0 and h0_any and not reset_sb[0, b]:
                        seed_b[r] = b
            off += m_j[j]
        plans.append((tok, seed_b))
    return m_j, plans


# ------------------------------------------------------------- device build

def _chunks(m):
    """Split m rows into balanced chunks of <= CH."""
    nch = (m + CH - 1) // CH
    base, rem = divmod(m, nch)
    out, off = [], 0
    for i in range(nch):
        f = base + (1 if i < rem else 0)
        out.append((off, f))
        off += f
    return out


def _build_nc(m_j, use_seed, j_pre):
    """j_pre: first pass whose gi comes from the fp16 presweep buffer."""
    Lmax = len(m_j)
    N_pad = sum(m_j)
    M_off = np.cumsum([0] + m_j)  # row offset of each pass block
    R0 = int(M_off[j_pre]) if j_pre < Lmax else N_pad  # presweep row range
    RN = N_pad - R0

    nc = bacc.Bacc("TRN2", target_bir_lowering=False, debug=False,
                   num_devices=NCORES)
    xT = nc.dram_tensor("xT", [DIM, N_pad], bf16, kind="ExternalInput")
    xT8 = nc.dram_tensor("xT8", [DIM, N_pad], fp8, kind="ExternalInput")
    # fp16 weights hold only the z and n gates; r is fp8 (DoubleRow)
    wihT = nc.dram_tensor("wihT", [DIM, 2 * UNITS], bf16, kind="ExternalInput")
    whhT = nc.dram_tensor("whhT", [UNITS, 2 * UNITS], bf16, kind="ExternalInput")
    wih8T = nc.dram_tensor("wih8T", [DIM, UNITS], fp8, kind="ExternalInput")
    whh8T = nc.dram_tensor("whh8T", [UNITS, UNITS], fp8, kind="ExternalInput")
    biases = nc.dram_tensor("biases", [UNITS, 4], f32, kind="ExternalInput")
    # per-unit bias rows for the packed tail's K=1 bias matmuls:
    # [b_r*W8SCALE | b_z | b_hhn]
    brow = nc.dram_tensor("brow", [1, 3 * UNITS], bf16, kind="ExternalInput")
    outT = nc.dram_tensor("outT", [UNITS, N_pad], bf16, kind="ExternalOutput")
    hseedT = hseed8T = None
    if use_seed:
        hseedT = nc.dram_tensor("hseedT", [UNITS, m_j[0]], bf16,
                                kind="ExternalInput")
        hseed8T = nc.dram_tensor("hseed8T", [UNITS, m_j[0]], fp8,
                                 kind="ExternalInput")

    Sig = mybir.ActivationFunctionType.Sigmoid
    Tanh = mybir.ActivationFunctionType.Tanh
    Copy = mybir.ActivationFunctionType.Copy
    ADD = mybir.AluOpType.add
    MULT = mybir.AluOpType.mult
    DR = mybir.MatmulPerfMode.DoubleRow
    RS = 1.0 / W8SCALE

    with tile.TileContext(nc) as tc:
        with (
            tc.tile_pool(name="wpool", bufs=1) as wpool,
            tc.tile_pool(name="xpool", bufs=2) as xpool,
            tc.tile_pool(name="hpool", bufs=1) as hpool,
            tc.tile_pool(name="spool", bufs=2) as spool,
            tc.tile_pool(name="ppool", bufs=2, space="PSUM") as ppool,
        ):
            wih_t = wpool.tile([P, CG, 2 * UNITS], bf16, tag="wih")
            whh_t = wpool.tile([P, CG, 2 * UNITS], bf16, tag="whh")
            wih8_t = wpool.tile([P, CG, UNITS], fp8, tag="wih8")
            whh8_t = wpool.tile([P, CG, UNITS], fp8, tag="whh8")

            x_tiles = {}
            x8_tiles = {}

            def get_x_tile(jj, ooff, ff):
                key = (jj, ooff)
                if key not in x_tiles:
                    x_t = xpool.tile([P, CG, CH], bf16, tag="x", name="x_t")
                    bb = int(M_off[jj]) + ooff
                    for c in range(CG):
                        nc.sync.dma_start(out=x_t[:, c, :ff],
                                          in_=xT[c * P:(c + 1) * P, bb: bb + ff])
                    x_tiles[key] = x_t
                return x_tiles[key]

            def get_x8_tile(jj, ooff, ff):
                key = (jj, ooff)
                if key not in x8_tiles:
                    x_t = xpool.tile([P, CG, CH], fp8, tag="x8", name="x8_t",
                                     bufs=1)
                    bb = int(M_off[jj]) + ooff
                    for c in range(CG):
                        nc.sync.dma_start(
                            out=x_t[:, c, :ff],
                            in_=xT8[c * P:(c + 1) * P, bb: bb + ff])
                    x8_tiles[key] = x_t
                return x8_tiles[key]

            # DMA emission order = need order: r-gate fp8 weights + fp8 x
            # (pass 0 starts with an r sweep), z-gate fp16 W_ih + fp16 x,
            # n-gate W_ih, small constants, second x chunk. W_hh and the
            # presweep inputs are emitted later.
            for c in range(CG):
                nc.sync.dma_start(out=wih8_t[:, c, :],
                                  in_=wih8T[c * P:(c + 1) * P, :])
            ch0 = _chunks(m_j[0])
            get_x8_tile(0, *ch0[0])
            for c in range(CG):
                nc.sync.dma_start(out=wih_t[:, c, 0:UNITS],
                                  in_=wihT[c * P:(c + 1) * P, 0:UNITS])
            get_x_tile(0, *ch0[0])
            for c in range(CG):
                nc.sync.dma_start(
                    out=wih_t[:, c, UNITS:2 * UNITS],
                    in_=wihT[c * P:(c + 1) * P, UNITS:2 * UNITS])
            b_t = wpool.tile([P, UG, 4], f32, tag="bias")
            for g in range(UG):
                nc.sync.dma_start(out=b_t[:, g, :], in_=biases[g * P:(g + 1) * P, :])
            brow_t = wpool.tile([1, 3 * UNITS], bf16, tag="brow")
            nc.sync.dma_start(out=brow_t[:, :], in_=brow[:, :])
            ones_t = wpool.tile([1, CH], bf16, tag="ones")
            nc.vector.memset(ones_t[:, :], 1.0)
            if len(ch0) > 1:
                get_x_tile(0, *ch0[1])

            def emit_whh():
                for c in range(CG):
                    nc.sync.dma_start(out=whh8_t[:, c, :],
                                      in_=whh8T[c * P:(c + 1) * P, :])
                for g in range(2):
                    for c in range(CG):
                        nc.sync.dma_start(
                            out=whh_t[:, c, g * UNITS:(g + 1) * UNITS],
                            in_=whhT[c * P:(c + 1) * P, g * UNITS:(g + 1) * UNITS])

            gi_pre = (wpool.tile([P, 3 * UG, RN], bf16, tag="gi_pre",
                                name="gi_pre")
                      if RN > 0 else None)
            xp_tiles = {}

            def get_xp_tiles():
                if "t" not in xp_tiles:
                    xp8_t = xpool.tile([P, CG, RN], fp8, tag="xpre8",
                                       bufs=1, name="xp8_t")
                    xp_t = xpool.tile([P, CG, RN], bf16, tag="xpre", bufs=1,
                                      name="xp_t")
                    for c in range(CG):
                        nc.sync.dma_start(out=xp8_t[:, c, :],
                                          in_=xT8[c * P:(c + 1) * P, R0:N_pad])
                        nc.sync.dma_start(out=xp_t[:, c, :],
                                          in_=xT[c * P:(c + 1) * P, R0:N_pad])
                    xp_tiles["t"] = (xp8_t, xp_t)
                return xp_tiles["t"]

            def emit_presweep_block(lo, hi):
                # gi for presweep rows [lo:hi) (relative to R0), all 24
                # (gate, unit-group) outputs. r-gate in fp8 DoubleRow, stored
                # *W8SCALE; n-gate copies fold in b_ihn via the activation.
                xp8_t, xp_t = get_xp_tiles()
                w = hi - lo
                with nc.named_scope(f"presweep{lo}"):
                    for gu in range(3 * UG):
                        ps_p = ppool.tile([P, CH], f32, tag="ps_gin",
                                          name="ps_pre")
                        if gu < UG:  # r gate: fp8 DoubleRow
                            for g in range(PG):
                                nc.tensor.matmul(
                                    ps_p[:, :w],
                                    lhsT=wih8_t[:, 2 * g:2 * g + 2,
                                                gu * P:(gu + 1) * P],
                                    rhs=xp8_t[:, 2 * g:2 * g + 2, lo:hi],
                                    start=(g == 0), stop=(g == PG - 1),
                                    perf_mode=DR)
                        else:
                            for c in range(CG):
                                nc.tensor.matmul(
                                    ps_p[:, :w],
                                    lhsT=wih_t[:, c, (gu - UG) * P:(gu - UG + 1) * P],
                                    rhs=xp_t[:, c, lo:hi],
                                    start=(c == 0), stop=(c == CG - 1))
                        if gu >= 2 * UG:  # n gate: fold b_ihn in here
                            nc.vector.tensor_scalar_add(
                                gi_pre[:, gu, lo:hi], ps_p[:, :w],
                                b_t[:, gu - 2 * UG, 2:3])
                        else:
                            nc.vector.tensor_copy(gi_pre[:, gu, lo:hi],
                                                  ps_p[:, :w])

            # presweep emission schedule: the block of rows consumed by pass
            # k is emitted at the start of pass max(1, k-3) — independent PE
            # work that fills inter-pass dependency bubbles.
            pre_blocks = {}
            for k in range(j_pre, Lmax):
                e = min(max(1, k - 3), k - 1)
                lo, hi = int(M_off[k]) - R0, int(M_off[k + 1]) - R0
                pre_blocks.setdefault(e, []).append((lo, hi))

            if use_seed:
                emit_whh()  # pass 0 already needs W_hh

            # ---------------------------------------------------- pass loop
            h_cur = None   # fp16 SBUF (P, CG, >=m_j[j]) input h for this pass
            h8_cur = None  # fp8 copy for the r-gate DoubleRow matmuls

            def out_dma(u, dcol, src):
                nc.sync.dma_start(out=outT[u * P:(u + 1) * P, dcol[0]:dcol[1]],
                                  in_=src)

            for j in range(Lmax):
                for (lo, hi) in pre_blocks.get(j, ()):
                    emit_presweep_block(lo, hi)
                scope = nc.named_scope(f"pass{j:02d}")
                scope.__enter__()
                m = m_j[j]
                m_next = m_j[j + 1] if j + 1 < Lmax else 0
                has_h = (j > 0) or use_seed
                pre = j >= j_pre
                base = int(M_off[j])
                packed = pre and has_h and m <= TAILM

                if packed:
                    p0 = base - R0
                    hout = hpool.tile([P, CG, m], bf16, tag=f"hbuf{j % 2}",
                                      name=f"hbuf{j}")
                    ps_r = ppool.tile([P, UG, TAILM], f32, tag="ps_r")
                    ps_z = ppool.tile([P, UG, TAILM], f32, tag="ps_z")
                    ps_n = ppool.tile([P, UG, TAILM], f32, tag="ps_ghn")
                    # K=1 bias matmuls seed each gate bank; only the very
                    # first write to a bank carries start=True (PSUM zero
                    # regions are whole banks).
                    for ps, gate in ((ps_r, 0), (ps_z, 1), (ps_n, 2)):
                        for u in range(UG):
                            nc.tensor.matmul(
                                ps[:, u, :m],
                                lhsT=brow_t[0:1, gate * UNITS + u * P:
                                            gate * UNITS + (u + 1) * P],
                                rhs=ones_t[0:1, :m],
                                start=(u == 0), stop=False,
                                skip_group_check=True)
                    # h matmuls, contraction-major so the earliest-produced
                    # h groups of the previous pass unblock the PE first.
                    for c in range(CG):
                        for u in range(UG):
                            nc.tensor.matmul(
                                ps_z[:, u, :m],
                                lhsT=whh_t[:, c, u * P:(u + 1) * P],
                                rhs=h_cur[:, c, :m],
                                start=False, stop=(c == CG - 1),
                                skip_group_check=True)
                            nc.tensor.matmul(
                                ps_n[:, u, :m],
                                lhsT=whh_t[:, c, UNITS + u * P:
                                           UNITS + (u + 1) * P],
                                rhs=h_cur[:, c, :m],
                                start=False, stop=(c == CG - 1),
                                skip_group_check=True)
                            if c % 2 == 0:
                                nc.tensor.matmul(
                                    ps_r[:, u, :m],
                                    lhsT=whh8_t[:, c:c + 2, u * P:(u + 1) * P],
                                    rhs=h8_cur[:, c:c + 2, :m],
                                    start=False, stop=(c == CG - 2),
                                    perf_mode=DR, skip_group_check=True)
                    # gates, full-width (all 8 unit groups per instruction)
                    r_sb = spool.tile([P, UG, TAILM], bf16, tag="r")
                    z_sb = spool.tile([P, UG, TAILM], bf16, tag="z")
                    n_sb = spool.tile([P, UG, TAILM], bf16, tag="n")
                    t2 = spool.tile([P, UG, TAILM], bf16, tag="t2")
                    arg = spool.tile([P, UG, TAILM], bf16, tag="d", name="arg")
                    nc.vector.tensor_add(r_sb[:, :, :m], ps_r[:, :, :m],
                                         gi_pre[:, 0:UG, p0:p0 + m])
                    nc.scalar.activation(r_sb[:, :, :m], r_sb[:, :, :m], Sig,
                                         scale=RS)
                    nc.vector.tensor_add(z_sb[:, :, :m], ps_z[:, :, :m],
                                         gi_pre[:, UG:2 * UG, p0:p0 + m])
                    nc.scalar.activation(z_sb[:, :, :m], z_sb[:, :, :m], Sig)
                    nc.vector.tensor_mul(t2[:, :, :m], ps_n[:, :, :m],
                                         r_sb[:, :, :m])
                    nc.vector.tensor_add(arg[:, :, :m], t2[:, :, :m],
                                         gi_pre[:, 2 * UG:3 * UG, p0:p0 + m])
                    nc.scalar.activation(n_sb[:, :, :m], arg[:, :, :m], Tanh)
                    d_sb = spool.tile([P, UG, TAILM], bf16, tag="d")
                    zd = spool.tile([P, UG, TAILM], bf16, tag="t2", name="zd")
                    nc.vector.tensor_sub(d_sb[:, :, :m], h_cur[:, :, :m],
                                         n_sb[:, :, :m])
                    nc.vector.tensor_mul(zd[:, :, :m], z_sb[:, :, :m],
                                         d_sb[:, :, :m])
                    nc.vector.tensor_add(hout[:, :, :m], n_sb[:, :, :m],
                                         zd[:, :, :m])
                    for u in range(UG):
                        out_dma(u, (base, base + m), hout[:, u, :m])
                    h8_next = None
                    if m_next > 0:
                        h8_next = hpool.tile([P, CG, m_next], fp8,
                                             tag=f"hbuf8_{j % 2}",
                                             name=f"hbuf8_{j}")
                        nc.vector.tensor_copy(h8_next[:, :, :m_next],
                                              hout[:, :, :m_next])
                    h_cur, h8_cur = hout, h8_next
                    scope.__exit__(None, None, None)
                    continue

                # ------------------------------------------- per-unit path
                hout = hpool.tile([P, CG, max(m_next, 1)], bf16,
                                  tag=f"hbuf{j % 2}", name=f"hbuf{j}")
                h8_next = (hpool.tile([P, CG, m_next], fp8,
                                      tag=f"hbuf8_{j % 2}", name=f"hbuf8_{j}")
                           if m_next > 0 else None)

                for ci, (off, f) in enumerate(_chunks(m)):
                    if not pre:
                        x_t = get_x_tile(j, off, f)
                        x8_t = get_x8_tile(j, off, f)
                    if j == 0 and use_seed:
                        hs_t = xpool.tile([P, CG, CH], bf16, tag="hseed",
                                          name="hs_t", bufs=1)
                        hs8_t = xpool.tile([P, CG, CH], fp8, tag="hseed8",
                                           name="hs8_t", bufs=1)
                        for c in range(CG):
                            nc.sync.dma_start(
                                out=hs_t[:, c, :f],
                                in_=hseedT[c * P:(c + 1) * P, off: off + f])
                            nc.sync.dma_start(
                                out=hs8_t[:, c, :f],
                                in_=hseed8T[c * P:(c + 1) * P, off: off + f])
                        h_in = lambda c: hs_t[:, c, :f]
                        h8_in = lambda g: hs8_t[:, 2 * g:2 * g + 2, :f]
                    elif has_h:
                        h_in = lambda c: h_cur[:, c, off: off + f]
                        h8_in = lambda g: h8_cur[:, 2 * g:2 * g + 2, off: off + f]
                    else:
                        h_in = h8_in = None
                    # presweep-relative row slice for this chunk
                    p0 = base + off - R0

                    def x_mms_r(ps, stop_at_end):
                        for g in range(PG):
                            nc.tensor.matmul(
                                ps[:, :f],
                                lhsT=wih8_t[:, 2 * g:2 * g + 2, u * P:(u + 1) * P],
                                rhs=x8_t[:, 2 * g:2 * g + 2, :f],
                                start=(g == 0),
                                stop=(stop_at_end and g == PG - 1),
                                perf_mode=DR)

                    def h_mms_r(ps, gs, do_start, do_stop):
                        gs = list(gs)
                        for g in gs:
                            nc.tensor.matmul(
                                ps[:, :f],
                                lhsT=whh8_t[:, 2 * g:2 * g + 2, u * P:(u + 1) * P],
                                rhs=h8_in(g),
                                start=(do_start and g == gs[0]),
                                stop=(do_stop and g == gs[-1]),
                                perf_mode=DR,
                                skip_group_check=True)

                    def x_mms(ps, gate, stop_at_end):
                        # gate: 0=z, 1=n in the fp16 weight tiles
                        for c in range(CG):
                            nc.tensor.matmul(
                                ps[:, :f],
                                lhsT=wih_t[:, c, gate * UNITS + u * P:
                                           gate * UNITS + (u + 1) * P],
                                rhs=x_t[:, c, :f],
                                start=(c == 0),
                                stop=(stop_at_end and c == CG - 1))

                    def h_mms(ps, gate, cs, do_start, do_stop):
                        cs = list(cs)
                        for c in cs:
                            nc.tensor.matmul(
                                ps[:, :f],
                                lhsT=whh_t[:, c, gate * UNITS + u * P:
                                           gate * UNITS + (u + 1) * P],
                                rhs=h_in(c),
                                start=(do_start and c == cs[0]),
                                stop=(do_stop and c == cs[-1]),
                                skip_group_check=True)

                    def h_out(u, n_sb, zd, sub):
                        """h' = n -/+ zd, written straight into the fp16 h
                        buffer (matmul input + DMA source) and, for rows with
                        no next pass, a recycled tail tile."""
                        op = nc.vector.tensor_sub if sub else nc.vector.tensor_add
                        pf = max(0, min(m_next - off, f))
                        if pf > 0:
                            op(hout[:, u, off:off + pf], n_sb[:, :pf], zd[:, :pf])
                            out_dma(u, (base + off, base + off + pf),
                                    hout[:, u, off:off + pf])
                            nc.vector.tensor_copy(h8_next[:, u, off:off + pf],
                                                  hout[:, u, off:off + pf])
                        if pf < f:
                            ht = spool.tile([P, CH], bf16, tag="n", name="h_tail")
                            op(ht[:, :f - pf], n_sb[:, pf:f], zd[:, pf:f])
                            out_dma(u, (base + off + pf, base + off + f),
                                    ht[:, :f - pf])

                    if j == 0 and not use_seed:
                        # pass 0: gate-major sweeps so the first matmuls only
                        # need the fp8 weights + fp8 x (first DMAs to land),
                        # and each sweep covers the next weight DMA's arrival.
                        if ci + 1 < len(ch0):
                            get_x_tile(0, *ch0[ci + 1])
                        r_sbs, z_sbs = [], []
                        for u in range(UG):
                            ps_r = ppool.tile([P, CH], f32, tag="ps_r")
                            x_mms_r(ps_r, stop_at_end=True)
                            r_sb = spool.tile([P, CH], bf16, tag="r0", bufs=8)
                            nc.scalar.activation(r_sb[:, :f], ps_r[:, :f], Sig,
                                                 bias=b_t[:, u, 0:1], scale=RS)
                            r_sbs.append(r_sb)
                        for u in range(UG):
                            ps_z = ppool.tile([P, CH], f32, tag="ps_z")
                            x_mms(ps_z, 0, stop_at_end=True)
                            z_sb = spool.tile([P, CH], bf16, tag="z0", bufs=8)
                            nc.scalar.activation(z_sb[:, :f], ps_z[:, :f], Sig,
                                                 bias=b_t[:, u, 1:2])
                            z_sbs.append(z_sb)
                        if ci == 0:
                            emit_whh()  # W_hh drains during pass-0 compute
                            if RN > 0:
                                get_xp_tiles()  # presweep x, ditto
                        for u in range(UG):
                            ps_gin = ppool.tile([P, CH], f32, tag="ps_gin")
                            x_mms(ps_gin, 1, stop_at_end=True)
                            # t2 = r*b_hhn + gi_n ; n = tanh(t2 + b_ihn)
                            t2 = spool.tile([P, CH], bf16, tag="t2")
                            nc.vector.scalar_tensor_tensor(
                                t2[:, :f], r_sbs[u][:, :f], b_t[:, u, 3:4],
                                ps_gin[:, :f], op0=MULT, op1=ADD)
                            n_sb = spool.tile([P, CH], bf16, tag="n")
                            nc.scalar.activation(n_sb[:, :f], t2[:, :f], Tanh,
                                                 bias=b_t[:, u, 2:3])
                            # h = (1-z)*n = n - z*n
                            zd = spool.tile([P, CH], bf16, tag="t2", name="zd")
                            nc.vector.tensor_mul(zd[:, :f], z_sbs[u][:, :f],
                                                 n_sb[:, :f])
                            h_out(u, n_sb, zd, sub=True)
                        continue

                    for u in range(UG):
                        ps_r = ppool.tile([P, CH], f32, tag="ps_r")
                        ps_z = ppool.tile([P, CH], f32, tag="ps_z")
                        if not pre:
                            ps_gin = ppool.tile([P, CH], f32, tag="ps_gin")
                        ps_ghn = (ppool.tile([P, CH], f32, tag="ps_ghn",
                                             name="ps_ghn")
                                  if has_h else None)

                        # For the first unit-tile of a chunk, defer every
                        # gate's last h-matmul to the end: it waits on the
                        # previous pass's last h cast, and deferring lets the
                        # other matmuls run during that wait.
                        split = has_h and u == 0 and off == 0
                        early = range(CG - 1) if split else range(CG)
                        early_g = range(PG - 1) if split else range(PG)
                        if not pre:
                            x_mms_r(ps_r, stop_at_end=not has_h)
                            if has_h:
                                h_mms_r(ps_r, early_g, False, not split)
                            x_mms(ps_z, 0, stop_at_end=not has_h)
                            if has_h:
                                h_mms(ps_z, 0, early, False, not split)
                            x_mms(ps_gin, 1, stop_at_end=True)
                            if has_h:
                                h_mms(ps_ghn, 1, early, True, not split)
                        else:
                            h_mms_r(ps_r, early_g, True, not split)
                            h_mms(ps_z, 0, early, True, not split)
                            h_mms(ps_ghn, 1, early, True, not split)
                        if split:
                            h_mms_r(ps_r, [PG - 1], False, True)
                            h_mms(ps_z, 0, [CG - 1], False, True)
                            h_mms(ps_ghn, 1, [CG - 1], False, True)

                        r_sb = spool.tile([P, CH], bf16, tag="r")
                        z_sb = spool.tile([P, CH], bf16, tag="z")
                        n_sb = spool.tile([P, CH], bf16, tag="n")
                        t2 = spool.tile([P, CH], bf16, tag="t2")
                        if pre:
                            # ps_r and gi_pre_r are both *W8SCALE; fold the
                            # rescale into the activation's scale.
                            nc.vector.tensor_add(r_sb[:, :f], ps_r[:, :f],
                                                 gi_pre[:, u, p0:p0 + f])
                            nc.scalar.activation(r_sb[:, :f], r_sb[:, :f], Sig,
                                                 bias=b_t[:, u, 0:1], scale=RS)
                            nc.vector.scalar_tensor_tensor(
                                z_sb[:, :f], ps_z[:, :f], b_t[:, u, 1:2],
                                gi_pre[:, UG + u, p0:p0 + f], op0=ADD, op1=ADD)
                            nc.scalar.activation(z_sb[:, :f], z_sb[:, :f], Sig)
                            nc.vector.scalar_tensor_tensor(
                                t2[:, :f], ps_ghn[:, :f], b_t[:, u, 3:4],
                                r_sb[:, :f], op0=ADD, op1=MULT)
                            arg = spool.tile([P, CH], bf16, tag="d", name="arg")
                            nc.vector.tensor_add(arg[:, :f], t2[:, :f],
                                                 gi_pre[:, 2 * UG + u, p0:p0 + f])
                            nc.scalar.activation(n_sb[:, :f], arg[:, :f], Tanh)
                        else:
                            nc.scalar.activation(r_sb[:, :f], ps_r[:, :f], Sig,
                                                 bias=b_t[:, u, 0:1], scale=RS)
                            nc.scalar.activation(z_sb[:, :f], ps_z[:, :f], Sig,
                                                 bias=b_t[:, u, 1:2])
                            # t2 = (ps_ghn + b_hhn) * r
                            nc.vector.scalar_tensor_tensor(
                                t2[:, :f], ps_ghn[:, :f], b_t[:, u, 3:4],
                                r_sb[:, :f], op0=ADD, op1=MULT)
                            arg = spool.tile([P, CH], bf16, tag="d", name="arg")
                            nc.vector.tensor_add(arg[:, :f], t2[:, :f],
                                                 ps_gin[:, :f])
                            nc.scalar.activation(n_sb[:, :f], arg[:, :f],
                                                 Tanh, bias=b_t[:, u, 2:3])
                        # h = n + z*(h_prev - n)
                        d_sb = spool.tile([P, CH], bf16, tag="d")
                        nc.vector.tensor_sub(d_sb[:, :f], h_in(u), n_sb[:, :f])
                        zd = spool.tile([P, CH], bf16, tag="t2", name="zd")
                        nc.vector.tensor_mul(zd[:, :f], z_sb[:, :f], d_sb[:, :f])
                        h_out(u, n_sb, zd, sub=False)
                    if not pre and (j, off) in x_tiles:
                        del x_tiles[(j, off)]  # consumed; let the slot recycle
                        del x8_tiles[(j, off)]
                h_cur = hout
                h8_cur = h8_next
                scope.__exit__(None, None, None)
    nc.compile()
    return nc


# ------------------------------------------------------------------- kernel

def kernel(x, h0, reset, W_ih, W_hh, b_ih, b_hh):
    global LAST_EXEC_NS
    x = np.asarray(x, np.float32)
    h0 = np.asarray(h0, np.float32)
    reset_sb = np.asarray(reset).reshape(SEQ, B).astype(bool)
    W_ih = np.asarray(W_ih, np.float32)
    W_hh = np.asarray(W_hh, np.float32)
    b_ih = np.asarray(b_ih, np.float32)
    b_hh = np.asarray(b_hh, np.float32)

    h0_any = bool(np.any(h0))
    m_j, plans = _build_plan(reset_sb, h0_any)
    N_pad = sum(m_j)

    b_sum = b_ih + b_hh
    biases = np.stack([b_sum[:UNITS], b_sum[UNITS:2 * UNITS],
                       b_ih[2 * UNITS:], b_hh[2 * UNITS:]], axis=1)
    biases = np.ascontiguousarray(biases, np.float32)
    brow = np.concatenate([b_sum[:UNITS] * W8SCALE, b_sum[UNITS:2 * UNITS],
                           b_hh[2 * UNITS:]])[None, :].astype(np.float16)
    e4m3 = ml_dtypes.float8_e4m3  # TRN FP8_EXP4 (max +-240)
    # fp16 weights: z and n gates only; r gate is fp8 e4m3 scaled by W8SCALE
    wihT = np.ascontiguousarray(W_ih[UNITS:].T).astype(np.float16)
    whhT = np.ascontiguousarray(W_hh[UNITS:].T).astype(np.float16)
    wih8T = np.ascontiguousarray((W_ih[:UNITS] * W8SCALE).T).astype(e4m3)
    whh8T = np.ascontiguousarray((W_hh[:UNITS] * W8SCALE).T).astype(e4m3)

    xf = x.reshape(SEQ * B, DIM)
    in_maps = []
    for c in range(NCORES):
        tok, seed_b = plans[c]
        real = tok >= 0
        xg = np.zeros((N_pad, DIM), np.float32)
        xg[real] = xf[tok[real]]
        xgT = np.ascontiguousarray(xg.T)
        m = {
            "xT": xgT.astype(np.float16),
            "xT8": np.clip(xgT, -240, 240).astype(e4m3),
            "wihT": wihT, "whhT": whhT,
            "wih8T": wih8T, "whh8T": whh8T, "biases": biases, "brow": brow,
        }
        if h0_any:
            hs = np.zeros((m_j[0], UNITS), np.float32)
            sreal = seed_b >= 0
            hs[sreal] = h0[seed_b[sreal]]
            hsT = np.ascontiguousarray(hs.T)
            m["hseedT"] = hsT.astype(np.float16)
            m["hseed8T"] = np.clip(hsT, -240, 240).astype(e4m3)
        in_maps.append(m)

    j_pre = 1
    while j_pre < len(m_j) and sum(m_j[j_pre:]) > CH:
        j_pre += 1
    nc = _build_nc(m_j, use_seed=h0_any, j_pre=j_pre)
    trace = os.environ.get("GRU_TRACE", "0") == "1"
    res = run_bass_kernel_spmd(nc, in_maps, list(range(NCORES)), trace=trace)
    LAST_EXEC_NS = res.exec_time_ns

    out = np.zeros((SEQ * B, UNITS), np.float32)
    for c in range(NCORES):
        tok, _ = plans[c]
        real = tok >= 0
        out[tok[real]] = res.results[c]["outT"].T[real].astype(np.float32)
    return out.reshape(SEQ, B, UNITS)
